# revision 15
# baseline (speedup 1.0000x reference)
"""Trainium2 Bass kernel for nn_CustomLoss_90537910600076 (nms_detection).

Computes, for in_signal/ref_signal [2048, 4096] f32:
  [total_loss, cosine_similarity, p2p_loss, mse_loss]  (f32 [4])

Pure data parallel over the batch dim across 8 NeuronCores (256 rows per
core, 2 blocks of 128 partitions). The device computes per-row sufficient
statistics; the host combines them:
  col0 dot    = sum(in*ref)
  col1 na2    = sum(in^2)
  col2 nb2    = sum(ref^2)
  col3 n_in   = #peaks(in, distance=20)
  col4 n_ref  = #peaks(ref, distance=20)
  col5 p2p    = sum((pk10(in) - pk10(ref))^2)

Peak mask per the reference: strict local maxima AND x >= window-max
(window 2d-1, SAME, -inf padded). Window max is exact-f32 via a pair-max
hierarchy + log-step max cascade in pair space, then combined with the
parity-dependent window-edge element.

Custom fused DVE ops (single pass each):
  LX  = select(x > nbmax, x, -FLT_MAX)        (strict local max, carries x)
  PK  = select(LX >= pooled19, LX, 0)         (= x at distance-10 peaks)
  CNT = sum((pk >= pooled39) & (pk != 0))     (exact distance-20 peak count;
        pk != 0 excludes non-peaks, and pooled39>=pooled19 makes every
        distance-20 peak a distance-10 peak so pk carries x there)
  SQDS accum = sum((pk_in - pk_ref)^2)
"""

import sys

if "/opt/trn_rl_repo" not in sys.path:
    sys.path.insert(0, "/opt/trn_rl_repo")

import numpy as np

B, L = 2048, 4096
NCORES = 8
ROWS_PER_CORE = B // NCORES      # 256
NBLK = ROWS_PER_CORE // 128      # 2
PADL = 20                        # left pad (>= 19, even)
PADR = 20
W = PADL + L + PADR              # 4136 (even)
NPAIR = W // 2                   # 2068
ALPHA, BETA = 1.0, 0.5
NEG = -3.0e38                    # stands in for -inf (finite keeps sim happy)

_CACHE = {}


def _mkap(bass, t, col_off, dims):
    """Custom view of a tile AP `t` ([128, ...]): keep the partition dim,
    replace free dims with explicit [step, count] pairs (element units),
    offset by col_off elements from t's start."""
    part = [list(d) for d in t.ap][0]
    return bass.AP(
        tensor=t.tensor,
        offset=int(t.offset) + int(col_off),
        ap=[part] + [[int(s), int(c)] for s, c in dims],
    )


def _register_custom_ops():
    """Define + self-pin the fused DVE ops, append them to dve_ops.OPS."""
    if "ops" in _CACHE:
        return _CACHE["ops"]
    import concourse.dve_ops as dve_ops
    from concourse.dve_spec import (
        Spec, Src0, Src1, C0, Zero, MaxNeg, lower, select, sq, ne,
        _has_src1,
    )
    from concourse.dve_uop import DveOpSpec
    from operator import add as _add

    FLT_MAX = np.float32(3.4028235e38)

    def _flat2(in0, in1):
        a = np.asarray(in0).reshape(np.asarray(in0).shape[0], -1)
        bb = np.asarray(in1).reshape(np.asarray(in1).shape[0], -1)
        return a, bb

    def _ref_lx(in0, in1, s0, s1, imm2):
        a, bb = _flat2(in0, in1)
        return np.where(a > bb, a, np.float32(-FLT_MAX)).astype(np.float32)

    def _ref_pk(in0, in1, s0, s1, imm2):
        a, bb = _flat2(in0, in1)
        return np.where(a >= bb, a, np.float32(0.0)).astype(np.float32)

    def _ref_cnt(in0, in1, s0, s1, imm2):
        a, bb = _flat2(in0, in1)
        b = ((a >= bb) & (a != 0.0)).astype(np.float32)
        return b, s0 + b.sum(axis=-1, keepdims=True)

    def _ref_sqds(in0, in1, s0, s1, imm2):
        a, bb = _flat2(in0, in1)
        b = ((a.astype(np.float32) - bb) ** 2).astype(np.float32)
        return b, s0 + b.sum(axis=-1, keepdims=True)

    specs = [
        ("ANT_NMS_LX", Spec(body=select(Src0 > Src1, Src0, MaxNeg), reference=_ref_lx)),
        ("ANT_NMS_PK", Spec(body=select(Src0 >= Src1, Src0, Zero), reference=_ref_pk)),
        (
            "ANT_NMS_CNT",
            Spec(
                body=(Src0 >= Src1) & ne(Src0, Zero),
                accum=_add,
                accum_init=C0,
                reference=_ref_cnt,
            ),
        ),
        (
            "ANT_NMS_SQDS",
            Spec(
                body=sq(Src0 - Src1),
                accum=_add,
                accum_init=C0,
                reference=_ref_sqds,
            ),
        ),
    ]

    ops = {}
    for i, (name, spec) in enumerate(specs):
        if any(op.name == name for op in dve_ops.OPS):
            ops[name] = next(op for op in dve_ops.OPS if op.name == name)
            continue
        row = dve_ops._CUSTOM_DVE_ROW_BASE + len(dve_ops.OPS)
        shas = {}
        for ver in ("v3", "v4"):
            r = DveOpSpec(
                name=name, opcode=row, uops=lower(spec, ver=ver),
                rd1_en=_has_src1(spec),
            )
            shas[ver] = r.sha(ver)
        op = dve_ops.DveOp(name, spec, subdim=False, uops_sha=shas)
        dve_ops.OPS.append(op)
        dve_ops.CUSTOM_DVE_SPECS[name] = spec
        ops[name] = op
    dve_ops._SUB_OPCODE_FOR_NAME = {
        op.name: dve_ops._CUSTOM_DVE_ROW_BASE + i for i, op in enumerate(dve_ops.OPS)
    }
    assert max(dve_ops._SUB_OPCODE_FOR_NAME.values()) < 0x20
    _CACHE["ops"] = ops
    return ops


def _build(repeat=1):
    """Build the SPMD program. `repeat` unrolls the whole 2-block body N
    times inside one NEFF (benchmarking only; outputs are just rewritten)."""
    import concourse.bass as bass
    import concourse.bacc as bacc
    import concourse.tile as tile
    import concourse.mybir as mybir
    from contextlib import ExitStack

    ops = _register_custom_ops()
    OP_LX, OP_PK, OP_CNT, OP_SQDS = (
        ops["ANT_NMS_LX"], ops["ANT_NMS_PK"], ops["ANT_NMS_CNT"], ops["ANT_NMS_SQDS"],
    )

    f32 = mybir.dt.float32
    Alu = mybir.AluOpType
    Act = mybir.ActivationFunctionType

    nc = bacc.Bacc("TRN2", target_bir_lowering=False)
    x_in = nc.dram_tensor("x_in", [ROWS_PER_CORE, L], f32, kind="ExternalInput").ap()
    x_ref = nc.dram_tensor("x_ref", [ROWS_PER_CORE, L], f32, kind="ExternalInput").ap()
    out_stats = nc.dram_tensor(
        "stats_out", [NBLK, 128, 6], f32, kind="ExternalOutput"
    ).ap()

    with ExitStack() as ctx:
        tc = ctx.enter_context(tile.TileContext(nc))
        sb = ctx.enter_context(tc.tile_pool(name="sb", bufs=1))
        ps = ctx.enter_context(tc.tile_pool(name="ps", bufs=1, space="PSUM"))

        NQ = NPAIR // 2  # quads per half (1034)

        for rep_b in range(repeat * NBLK):
            b = rep_b % NBLK
            rows = slice(b * 128, (b + 1) * 128)

            # SIG is double-buffered so the next block's loads overlap compute
            SIG = sb.tile([128, 2, W], f32, tag="SIG", bufs=2, name=f"SIG{rep_b}")
            PA = sb.tile([128, 2, NPAIR], f32, tag="PA", name=f"PA{rep_b}")
            PB = sb.tile([128, 2, NPAIR], f32, tag="PB", name=f"PB{rep_b}")
            PC = sb.tile([128, 2, NPAIR], f32, tag="PC", name=f"PC{rep_b}")
            E1 = sb.tile([128, 1040], f32, tag="E1", name=f"E1{rep_b}")
            E2 = sb.tile([128, 1040], f32, tag="E2", name=f"E2{rep_b}")
            E3 = sb.tile([128, 1040], f32, tag="E3", name=f"E3{rep_b}")
            NB = sb.tile([128, L], f32, tag="NB", name=f"NB{rep_b}")
            LX = sb.tile([128, L], f32, tag="LX", name=f"LX{rep_b}")
            PK = sb.tile([128, 2, L], f32, tag="PK", name=f"PK{rep_b}")
            STATS = sb.tile([128, 8], f32, tag="STATS", name=f"STATS{rep_b}")
            ACTS = ps.tile([128, L], f32, tag="ACTS", name=f"ACTS{rep_b}")

            sig_h = int(SIG.ap[1][0])  # per-half element strides
            pa_h = int(PA.ap[1][0])
            pb_h = int(PB.ap[1][0])
            pc_h = int(PC.ap[1][0])

            # --- load + pad init -------------------------------------------
            nc.sync.dma_start(out=SIG[:, 0, PADL : PADL + L], in_=x_in[rows, :])
            nc.sync.dma_start(out=SIG[:, 1, PADL : PADL + L], in_=x_ref[rows, :])
            nc.gpsimd.memset(SIG[:, :, 0:PADL], NEG)
            nc.gpsimd.memset(SIG[:, :, W - PADR : W], NEG)

            def tmax(out, i0, i1):
                nc.vector.tensor_tensor(out=out, in0=i0, in1=i1, op=Alu.max)

            # --- window-max hierarchy (pair + quad levels), per half -------
            # p[i]   = max over pair i (2 elems)
            # d1[i]  = max(p[i], p[i+1]); p2[u] := d1[2u] = max over quad u
            # e1/e2/e3 cover 2/4/8 quads; Q9 covers 9 quads
            # q9[i]  = max over pairs [i, i+8]   (distance-10 window body)
            # q19[i] = max over pairs [i, i+18]  (distance-20 window body)
            for h in range(2):
                tmax(
                    PA[:, h, 0:NPAIR],
                    _mkap(bass, SIG, h * sig_h + 0, [[2, NPAIR]]),
                    _mkap(bass, SIG, h * sig_h + 1, [[2, NPAIR]]),
                )
                tmax(PB[:, h, 0:NPAIR-1], PA[:, h, 0:NPAIR-1], PA[:, h, 1:NPAIR])
                # e1[u] = max(d1[2u], d1[2u+2])
                tmax(
                    E1[:, 0:NQ-2],
                    _mkap(bass, PB, h * pb_h + 0, [[2, NQ - 2]]),
                    _mkap(bass, PB, h * pb_h + 2, [[2, NQ - 2]]),
                )
                tmax(E2[:, 0:NQ-4], E1[:, 0:NQ-4], E1[:, 2:NQ-2])
                tmax(E3[:, 0:NQ-8], E2[:, 0:NQ-8], E2[:, 4:NQ-4])
                # Q9 -> E1 (e1 dead after e2)
                tmax(E1[:, 0:NQ-9], E3[:, 0:NQ-9], E3[:, 1:NQ-8])
                # q9[2u]   = max(e2[u], p[2u+8]);  q9[2u+1] = max(p[2u+1], e2[u+1])
                tmax(
                    _mkap(bass, PC, h * pc_h + 0, [[2, NQ - 7]]),
                    E2[:, 0:NQ-7],
                    _mkap(bass, PA, h * pa_h + 8, [[2, NQ - 7]]),
                )
                tmax(
                    _mkap(bass, PC, h * pc_h + 1, [[2, NQ - 7]]),
                    _mkap(bass, PA, h * pa_h + 1, [[2, NQ - 7]]),
                    E2[:, 1:NQ-6],
                )
                # q19[2u]  = max(Q9[u], p[2u+18]); q19[2u+1] = max(p[2u+1], Q9[u+1])
                # (q19 -> PB; d1 is dead after e1)
                tmax(
                    _mkap(bass, PB, h * pb_h + 0, [[2, NQ - 9]]),
                    E1[:, 0:NQ-9],
                    _mkap(bass, PA, h * pa_h + 18, [[2, NQ - 9]]),
                )
                tmax(
                    _mkap(bass, PB, h * pb_h + 1, [[2, NQ - 10]]),
                    _mkap(bass, PA, h * pa_h + 1, [[2, NQ - 10]]),
                    E1[:, 1:NQ-9],
                )

            # --- masks via chained selects (no pooled tiles materialized) --
            for h in range(2):
                sig_data = _mkap(bass, SIG, h * sig_h + PADL, [[1, L]])

                # nbmax = max(x[j-1], x[j+1])
                tmax(
                    NB[:, 0:L],
                    _mkap(bass, SIG, h * sig_h + PADL - 1, [[1, L]]),
                    _mkap(bass, SIG, h * sig_h + PADL + 1, [[1, L]]),
                )
                # LX = x where strict local max else -FLT_MAX; kill edges
                nc.vector._custom_dve(OP_LX, out=LX[:, 0:L], in0=sig_data, in1=NB[:, 0:L])
                nc.vector.memset(_mkap(bass, LX, 0, [[L - 1, 2]]), NEG)
                # pk1 = LX where LX >= q9[6 + j//2] else 0   (window-19 body)
                nc.vector._custom_dve(
                    OP_PK,
                    out=NB[:, 0:L],
                    in0=LX[:, 0:L],
                    in1=_mkap(bass, PC, h * pc_h + 6, [[1, L // 2], [0, 2]]),
                )
                # pk = pk1 where pk1 >= window-19 edge elem else 0
                nc.vector._custom_dve(
                    OP_PK,
                    out=PK[:, h, 0:L],
                    in0=NB[:, 0:L],
                    in1=_mkap(bass, SIG, h * sig_h + 11, [[2, L // 2], [19, 2]]),
                )
                # c1 = pk where pk >= q19[1 + j//2] else 0   (window-39 body)
                nc.vector._custom_dve(
                    OP_PK,
                    out=NB[:, 0:L],
                    in0=PK[:, h, 0:L],
                    in1=_mkap(bass, PB, h * pb_h + 1, [[1, L // 2], [0, 2]]),
                )
                # n20 = #(c1 >= window-39 edge elem & c1 != 0) -> stats col 3+h
                nc.vector._custom_dve(
                    OP_CNT,
                    out=LX[:, 0:L],
                    in0=NB[:, 0:L],
                    in1=_mkap(bass, SIG, h * sig_h + 1, [[2, L // 2], [39, 2]]),
                    s0=0.0,
                    accum_out=STATS[:, 3 + h : 4 + h],
                )
                # sum of squares of this half -> stats col 1+h  (ACT engine)
                nc.scalar.activation(
                    out=ACTS[:, 0:L],
                    in_=sig_data,
                    func=Act.Square,
                    accum_out=STATS[:, 1 + h : 2 + h],
                )

            # dot = sum(in*ref) -> stats col 0 (custom-DVE TTR; the stock
            # InstTensorTensorReduce wedges the device on this runtime)
            from concourse.dve_ops import TENSOR_TENSOR_REDUCE as OP_TTR

            nc.vector._custom_dve(
                OP_TTR,
                out=NB[:, 0:L],
                in0=_mkap(bass, SIG, 0 * sig_h + PADL, [[1, L]]),
                in1=_mkap(bass, SIG, 1 * sig_h + PADL, [[1, L]]),
                s0=0.0,
                s1=1.0,
                accum_out=STATS[:, 0:1],
            )
            # p2p = sum((pk_in - pk_ref)^2) -> stats col 5
            nc.vector._custom_dve(
                OP_SQDS,
                out=NB[:, 0:L],
                in0=PK[:, 0, 0:L],
                in1=PK[:, 1, 0:L],
                s0=0.0,
                accum_out=STATS[:, 5:6],
            )

            nc.sync.dma_start(out=out_stats[b, :, :], in_=STATS[:, 0:6])

    nc.compile()
    return nc


def _get_nc():
    if "nc" not in _CACHE:
        _CACHE["nc"] = _build()
    return _CACHE["nc"]


def run_device(in_signal, ref_signal):
    """Run the SPMD kernel; returns per-row stats [B, 6] float32."""
    from concourse.bass_utils import run_bass_kernel_spmd

    nc = _get_nc()
    in_maps = []
    for c in range(NCORES):
        r = slice(c * ROWS_PER_CORE, (c + 1) * ROWS_PER_CORE)
        in_maps.append(
            {
                "x_in": np.ascontiguousarray(in_signal[r], dtype=np.float32),
                "x_ref": np.ascontiguousarray(ref_signal[r], dtype=np.float32),
            }
        )
    res = run_bass_kernel_spmd(nc, in_maps, list(range(NCORES))).results
    stats = np.concatenate(
        [np.asarray(res[c]["stats_out"]).reshape(ROWS_PER_CORE, 6) for c in range(NCORES)],
        axis=0,
    )
    return stats


def finalize(stats):
    """Host combine of per-row stats -> [4] f32 output."""
    dot = stats[:, 0].astype(np.float64)
    na2 = stats[:, 1].astype(np.float64)
    nb2 = stats[:, 2].astype(np.float64)
    n_in = stats[:, 3]
    n_ref = stats[:, 4]
    p2p_sum = stats[:, 5].astype(np.float64)

    sqsum = na2 + nb2 - 2.0 * dot
    mse_i = sqsum / L
    mse_loss = sqsum.sum() / (B * L)
    cosine = (dot / np.sqrt(na2 * nb2)).mean()
    p2p_i = p2p_sum / L
    p2p_loss = p2p_i.sum()
    custom = np.where(n_in != n_ref, mse_i * ALPHA, p2p_i * BETA).sum()
    total = mse_loss + custom
    return np.array([total, cosine, p2p_loss, mse_loss], dtype=np.float32)


def kernel(in_signal, ref_signal):
    stats = run_device(np.asarray(in_signal), np.asarray(ref_signal))
    return finalize(stats)
